# revision 30
# baseline (speedup 1.0000x reference)
"""Causal multi-head self-attention on 8 Trainium2 NeuronCores (v2).

Problem: B=4, S=2048, D_MODEL=2048, H=16 heads, d_k=128, RoPE, causal
softmax, fp32 I/O.

Sharding: 8 cores = 4 batches x 2 head-groups (8 heads each).  Each core
computes QKV projections for its head group (weights sharded by output
rows), RoPE, head-local causal attention, and a partial o_proj over its
1024 input features.  The host sums the two partial o_proj outputs per
batch.

v2 changes vs v1:
- All matmul operands bf16 (same 1 cyc/row PE rate as f32r, but no
  4x penalty on <256-wide moving suffixes, half the DMA bytes, half
  the SBUF), fp32 PSUM accumulation throughout.  fp8 was evaluated and
  rejected: probs overflow fp8e4's range on the real logit tails (std
  1.41, max 10.4 without max-subtraction), and fp8 V / o_proj measured
  over the 2e-2 error bar.
- Q/K/V never round-trip through DRAM: RoPE'd QT/KT and V (all bf16)
  stay SBUF-resident for the whole kernel.
- kc-chunks are processed in PAIRS; exp for non-diagonal pairs is one
  fused ACT op over both chunks' PSUM banks.
- o_proj matmuls (bf16) are spliced between attention pairs one q-block
  behind, so the PE never idles on the ACT exp latency; o_proj PSUM
  eviction runs on the (idle) DVE instead of ACT.
- x / weights stream on both HWDGE queues (SP + ACT) round-robin; out
  rides the HWDGE queues too (SWDGE ucode is ~1us/op on Pool).
- out is written bf16 (host upcasts; threshold is 2e-2 >> bf16 rounding).
"""

import sys

for _p in ("/opt/trn_rl_repo", "/root/.axon_site/_ro/trn_rl_repo"):
    if _p not in sys.path:
        sys.path.insert(0, _p)

import numpy as np
import ml_dtypes

import concourse.bacc as bacc
import concourse.mybir as mybir
import concourse.tile as tile

F32 = mybir.dt.float32
BF16 = mybir.dt.bfloat16
FP8 = mybir.dt.float8e4
EXPF = mybir.ActivationFunctionType.Exp
COPYF = mybir.ActivationFunctionType.Copy
MUL = mybir.AluOpType.mult
ADD = mybir.AluOpType.add
SUB = mybir.AluOpType.subtract
DR = mybir.MatmulPerfMode.DoubleRow

D_MODEL = 2048
NUM_HEADS = 16
D_K = 128
ROPE_THETA = 10000.0
B = 4
S = 2048
N_CORES = 8
GROUPS = 2  # head groups (tensor parallel factor)
H_LOC = NUM_HEADS // GROUPS  # heads per core

# Global bias folded into the exp (cancels in the softmax ratio).  The
# real logits here have std ~1.41 and max ~10.4: the fp8e4 dynamic range
# (~11.7 nats) cannot span the ~19-nat logit spread without max
# subtraction, so probs stay bf16 (no overflow: bf16 spans e+-88 nats).
EXP_BIAS = 0.0

# fp8 probs overflowed to Inf on the real logit tails (max logit 10.4 >
# ln(240)+bias headroom); V / o_proj in fp8 measured over the 2e-2 bar.
# Everything stays bf16.
USE_FP8_PROBS = False
USE_DR = False  # fp8 DoubleRow denominator requires fp8 probs


def build_nc(D=D_MODEL, S_=S, H_loc=H_LOC, QB=512):
    """Build the per-core Bass program. Parametric for small-size sim tests."""
    P = 128
    DK = 128
    HH = DK // 2
    E = H_loc * DK  # local qkv output features
    KCN = D // P  # contraction chunks for projections
    NSB = S_ // P  # 128-token blocks
    NQB = S_ // QB  # q blocks in attention
    NDIAG = QB // P  # diagonal 128-k chunks per q block (4)
    NST = max(1, S_ // 512)  # 512-wide s tiles in projections
    STW = S_ // NST
    NTW = min(512, D)  # o_proj output tile width
    NTN = D // NTW
    SCALE = 1.0 / float(np.sqrt(DK))

    nc = bacc.Bacc("TRN2", target_bir_lowering=False, debug=False,
                   num_devices=N_CORES)

    if (F32, EXP_BIAS) not in nc.const_aps.aps:
        # register the exp-bias const AP (only 0.0/1.0 ship by default)
        _bias_t = nc.alloc_sbuf_tensor("const-f32-expbias", [128, 1], F32)
        nc.gpsimd.memset(_bias_t.ap(), EXP_BIAS)
        nc.const_aps.aps[(F32, EXP_BIAS)] = _bias_t.ap()
        nc.all_engine_barrier()

    xT = nc.dram_tensor("xT", [D, S_], BF16, kind="ExternalInput")
    wqT = nc.dram_tensor("wqT", [D, E], BF16, kind="ExternalInput")
    wkT = nc.dram_tensor("wkT", [D, E], BF16, kind="ExternalInput")
    wvT = nc.dram_tensor("wvT", [D, E], BF16, kind="ExternalInput")
    woT = nc.dram_tensor("woT", [E, D], BF16, kind="ExternalInput")
    # RoPE tables for the DE-INTERLEAVED head layout (even dims in rows
    # 0..63, odd dims in rows 64..127; wq/wk columns permuted host-side,
    # leaving Q.K dot products invariant).  Duplicated to full d_k height
    # so both halves have base-0 AND base-64 slices.
    cosH = nc.dram_tensor("cosH", [DK, S_], BF16, kind="ExternalInput")
    sinH = nc.dram_tensor("sinH", [DK, S_], BF16, kind="ExternalInput")
    # masks[:, 0:128] = causal triangle; masks[:, 128:384] = [zeros | tri]
    # (the odd chunk of a diagonal pair needs a fully-invalid 128 block
    # followed by its triangle).
    EDT = FP8 if USE_FP8_PROBS else BF16  # probs dtype
    masks = nc.dram_tensor("masks", [P, 3 * P], EDT, kind="ExternalInput")
    # padded to [P, 2, 16] so the dual-fp8 ldweights sees a 16B-aligned
    # even step on the k-tile dim (s3_lw_dual_fp8_restrictions)
    ones_in = nc.dram_tensor("ones", [P, 32], FP8, kind="ExternalInput")
    out = nc.dram_tensor("out", [S_, D], BF16, kind="ExternalOutput")

    xT_t = xT.rearrange("(kc p) s -> p kc s", p=P)
    wT_t = {
        "q": wqT.rearrange("(kc p) e -> p kc e", p=P),
        "k": wkT.rearrange("(kc p) e -> p kc e", p=P),
        "v": wvT.rearrange("(kc p) e -> p kc e", p=P),
    }
    woT_t = woT.rearrange("(ec p) n -> p ec n", p=P)

    with tile.TileContext(nc) as tc:
        with (
            tc.tile_pool(name="const", bufs=1) as const,
            tc.tile_pool(name="qkv", bufs=1) as qkvp,
        ):
            mask_sb = const.tile([P, 3 * P], EDT)
            ones_sb = const.tile([P, 2, 16], FP8)
            cos_sb = const.tile([DK, S_], BF16)
            sin_sb = const.tile([DK, S_], BF16)
            # const loads ride the (idle) SWDGE so the HWDGE queues start
            # streaming x immediately
            nc.gpsimd.dma_start(mask_sb[:], masks[:])
            nc.gpsimd.dma_start(ones_sb[:], ones_in[:])

            # Q/K head-transposed [dk, h, s] bf16; V [s, sb, h, dk] bf16 --
            # SBUF-resident for the whole kernel (no DRAM roundtrip).
            qt_sbuf = qkvp.tile([DK, H_loc, S_], BF16)
            kt_sbuf = qkvp.tile([DK, H_loc, S_], BF16)
            v_sbuf = qkvp.tile([P, NSB, H_loc, DK], BF16)

            queues = [nc.sync, nc.scalar]  # two HWDGE queues

            # ---------------- Phase 1: projections + RoPE -----------------
            with tc.tile_pool(name="xres", bufs=1) as xres:
                x_res = xres.tile([P, KCN, S_], BF16)

                # --- Phase 1b: V via x-stationary matmuls -> v_sbuf fp8 ---
                with (
                    tc.tile_pool(name="wv", bufs=1) as wvp,
                    tc.tile_pool(name="v_ps", bufs=8, space="PSUM") as v_ps,
                ):
                    EH = min(512, E)
                    NH_H = EH // DK  # heads per feature-half
                    gsz = min(4, NSB)
                    for half in range(E // EH):
                        e_lo = half * EH
                        h_lo = e_lo // DK
                        wv_res = wvp.tile([P, KCN, EH], BF16, tag="wv",
                                          name=f"wv_{half}")
                        if half != 0:
                            for kc in range(KCN):
                                queues[kc % 2].dma_start(
                                    wv_res[:, kc],
                                    wT_t["v"][:, kc, e_lo:e_lo + EH],
                                )
                        sb0 = 0
                        for gi in range(NSB // gsz):
                            psv = [
                                v_ps.tile([P, EH], F32, tag="vps",
                                          name=f"vps_{half}_{gi}_{i}")
                                for i in range(gsz)
                            ]
                            for kc in range(KCN):
                                if half == 0 and gi == 0:
                                    # stream x + wv together on both queues;
                                    # first x chunk split across both queues
                                    # for startup latency
                                    if kc == 0:
                                        hw = S_ // 2
                                        queues[0].dma_start(
                                            x_res[:, 0, :hw],
                                            xT_t[:, 0, :hw])
                                        queues[1].dma_start(
                                            x_res[:, 0, hw:],
                                            xT_t[:, 0, hw:])
                                        nc.gpsimd.dma_start(
                                            wv_res[:, 0],
                                            wT_t["v"][:, 0,
                                                      e_lo:e_lo + EH])
                                    else:
                                        queues[kc % 2].dma_start(
                                            x_res[:, kc], xT_t[:, kc])
                                        queues[(kc + 1) % 2].dma_start(
                                            wv_res[:, kc],
                                            wT_t["v"][:, kc,
                                                      e_lo:e_lo + EH])
                                for i in range(gsz):
                                    sb_i = sb0 + i
                                    nc.tensor.matmul(
                                        psv[i][:],
                                        x_res[:, kc, sb_i * P:(sb_i + 1) * P],
                                        wv_res[:, kc],
                                        start=(kc == 0),
                                        stop=(kc == KCN - 1),
                                    )
                            for i in range(gsz):
                                sb_i = sb0 + i
                                nc.scalar.activation(
                                    v_sbuf[:, sb_i, h_lo:h_lo + NH_H, :],
                                    psv[i][:].rearrange(
                                        "p (a b) -> p a b", a=NH_H),
                                    COPYF,
                                )
                            sb0 += gsz
                    # trig tables land long before the first RoPE needs them
                    queues[0].dma_start(cos_sb[:], cosH[:])
                    queues[1].dma_start(sin_sb[:], sinH[:])

                # --- Phase 1a: Q/K head-transposed projections + RoPE ---
                with (
                    tc.tile_pool(name="wslice", bufs=6) as wslice,
                    tc.tile_pool(name="qk_ps", bufs=2, space="PSUM") as qk_ps,
                    tc.tile_pool(name="rawp", bufs=2) as rawp,
                    tc.tile_pool(name="ropet", bufs=2) as ropet,
                ):
                    qi = 0
                    for t in ("q", "k"):
                        for h in range(H_loc):
                            pgrp = qk_ps.tile([P, NST, STW], F32, tag="qk",
                                              name=f"pg_{t}_{h}")
                            for kc in range(KCN):
                                w_sl = wslice.tile([P, DK], BF16, tag="wsl")
                                queues[qi % 2].dma_start(
                                    w_sl[:],
                                    wT_t[t][:, kc, h * DK:(h + 1) * DK])
                                qi += 1
                                for st in range(NST):
                                    nc.tensor.matmul(
                                        pgrp[:, st],
                                        w_sl[:],
                                        x_res[:, kc,
                                              st * STW:(st + 1) * STW],
                                        start=(kc == 0),
                                        stop=(kc == KCN - 1),
                                    )
                            # single ACT eviction to bf16, then RoPE on DVE
                            # (bf16 = 2x DVE rate) writing straight into
                            # qt_sbuf/kt_sbuf.
                            raw = rawp.tile([DK, S_], BF16, tag="raw")
                            raw_v = raw[:].rearrange("p (a b) -> p a b",
                                                     b=STW)
                            nc.scalar.activation(raw_v, pgrp[:], COPYF)
                            dst = qt_sbuf if t == "q" else kt_sbuf
                            dstE = dst[:HH, h]
                            dstO = dst[HH:, h]
                            tmp = ropet.tile([HH, S_], BF16, tag="tmp")
                            rotO = ropet.tile([HH, S_], BF16, tag="rotO")
                            # rot_E = E*cos - O*sin (in-place in qt/kt)
                            nc.vector.tensor_tensor(
                                dstE, raw[:HH], cos_sb[:HH], MUL)
                            nc.vector.tensor_tensor(
                                tmp[:], raw[HH:], sin_sb[HH:], MUL)
                            nc.vector.tensor_tensor(
                                dstE, dstE, tmp[:], SUB)
                            # rot_O = E*sin + O*cos (via base-0 temps; the
                            # SB-SB input pair must share a base partition)
                            nc.vector.tensor_tensor(
                                rotO[:], raw[:HH], sin_sb[:HH], MUL)
                            nc.vector.tensor_tensor(
                                tmp[:], raw[HH:], cos_sb[HH:], MUL)
                            nc.vector.tensor_tensor(
                                dstO, rotO[:], tmp[:], ADD)

            # ---------- Phase 2+3: attention with o_proj spliced ----------
            with (
                tc.tile_pool(name="attnTp", bufs=1) as attnTp,
                tc.tile_pool(name="wop", bufs=1) as wop,
                tc.tile_pool(name="expt", bufs=4) as expt,
                tc.tile_pool(name="invp", bufs=2) as invp,
                tc.tile_pool(name="osb", bufs=3) as osbp,
                tc.tile_pool(name="sc_ps", bufs=2, space="PSUM") as sc_ps,
                tc.tile_pool(name="den_ps", bufs=1, space="PSUM") as den_ps,
                tc.tile_pool(name="pv_ps", bufs=1, space="PSUM") as pv_ps,
                tc.tile_pool(name="op_ps", bufs=2, space="PSUM") as op_ps,
            ):
                attnT = attnTp.tile([DK, H_loc, S_], BF16)
                wo_sb = wop.tile([P, H_loc, D], BF16)
                for ec in range(H_loc):
                    queues[ec % 2].dma_start(wo_sb[:, ec], woT_t[:, ec])

                mask_tri = mask_sb[:, 0:P]
                mask_zt = mask_sb[:, P:3 * P]

                # o_proj work list; units are spliced between attention
                # pairs one q-block behind so the PE never waits on exp.
                class OProj:
                    def __init__(self):
                        self.units = []
                        self.credit = 0.0

                    def add_qb(self, qb, tail=False):
                        for sb_i in range(qb * (QB // P), (qb + 1) * (QB // P)):
                            for nt in range(NTN):
                                ps = op_ps.tile([P, NTW], F32, tag="op",
                                                name=f"op_{sb_i}_{nt}")

                                for ec0 in range(0, H_loc, 2):
                                    def mm(sb_i=sb_i, nt=nt, ps=ps, ec0=ec0):
                                        for ec in (ec0, ec0 + 1):
                                            nc.tensor.matmul(
                                                ps[:],
                                                attnT[:, ec,
                                                      sb_i * P:(sb_i + 1) * P],
                                                wo_sb[:, ec,
                                                      nt * NTW:(nt + 1) * NTW],
                                                start=(ec == 0),
                                                stop=(ec == H_loc - 1),
                                            )
                                    self.units.append(mm)

                                # evict on DVE (ACT is busy with exp); in the
                                # tail drain ACT is idle, so alternate
                                alt = (sb_i + nt) % 2
                                def ev(sb_i=sb_i, nt=nt, ps=ps, alt=alt,
                                       tail=tail):
                                    o_nt = osbp.tile([P, NTW], BF16,
                                                     tag="osb")
                                    if tail and alt:
                                        nc.scalar.activation(
                                            o_nt[:], ps[:], COPYF)
                                    else:
                                        nc.vector.tensor_scalar_mul(
                                            o_nt[:], ps[:], 1.0)
                                    # out rides the HWDGE queues (idle in
                                    # the attention phase)
                                    queues[alt].dma_start(
                                        out[sb_i * P:(sb_i + 1) * P,
                                            nt * NTW:(nt + 1) * NTW],
                                        o_nt[:],
                                    )
                                self.units.append(ev)

                    def step(self, quota):
                        self.credit += quota
                        while self.credit >= 1.0 and self.units:
                            self.units.pop(0)()
                            self.credit -= 1.0

                    def drain(self):
                        while self.units:
                            self.units.pop(0)()

                oproj = OProj()

                for qb in range(NQB):
                    npair = (qb + 1) * NDIAG // 2
                    # warmup-skip: no filler during the first head of a qb
                    # (its attnT inputs are still in the normalize chain of
                    # the previous qb), redistribute over the rest
                    quota = (len(oproj.units) / float((H_loc - 1) * npair)
                             if qb > 0 and H_loc > 1 else 0.0)
                    for h in range(H_loc):
                        hquota = quota if (qb > 0 and h > 0) else 0.0
                        ps_d = den_ps.tile([1, QB], F32, tag="den")
                        ps_o = pv_ps.tile([P, QB], F32, tag="pv")

                        def off_of(m):
                            dj = m - qb * (NDIAG // 2)
                            return 2 * P * dj if dj > 0 else 0

                        def scores_exp(m):
                            off = off_of(m)
                            dj = m - qb * (NDIAG // 2)
                            e_pair = expt.tile([P, 2, QB], EDT, tag="e",
                                               name=f"e_{h}_{qb}_{m}")
                            ps_s = sc_ps.tile([P, 2, QB], F32, tag="sc",
                                              name=f"ss_{h}_{qb}_{m}")
                            for i in (0, 1):
                                kc = 2 * m + i
                                nc.tensor.matmul(
                                    ps_s[:, i, off:],
                                    kt_sbuf[:, h, kc * P:(kc + 1) * P],
                                    qt_sbuf[:, h, qb * QB + off:(qb + 1) * QB],
                                    start=True, stop=True,
                                )
                            if dj < 0:
                                # full pair: one fused exp over both chunks
                                nc.scalar.activation(
                                    e_pair[:].rearrange("p a b -> p (a b)"),
                                    ps_s[:].rearrange("p a b -> p (a b)"),
                                    EXPF, scale=SCALE, bias=EXP_BIAS)
                            else:
                                for i in (0, 1):
                                    nc.scalar.activation(
                                        e_pair[:, i, off:], ps_s[:, i, off:],
                                        EXPF, scale=SCALE, bias=EXP_BIAS)
                                # even chunk: triangle at [off, off+P);
                                # odd chunk: zeros [off,off+P) + triangle
                                nc.vector.tensor_tensor(
                                    e_pair[:, 0, off:off + P],
                                    e_pair[:, 0, off:off + P],
                                    mask_tri, MUL)
                                nc.vector.tensor_tensor(
                                    e_pair[:, 1, off:off + 2 * P],
                                    e_pair[:, 1, off:off + 2 * P],
                                    mask_zt, MUL)
                            return e_pair

                        def denom_pv(m, e_pair):
                            off = off_of(m)
                            st, sp = (m == 0), (m == npair - 1)
                            if USE_DR and USE_FP8_PROBS:
                                # fp8 DoubleRow denominator: 256-contraction
                                # per pass over the chunk pair
                                nc.tensor.matmul(
                                    ps_d[:, off:], ones_sb[:, :, 0:1],
                                    e_pair[:, :, off:],
                                    start=st, stop=sp, perf_mode=DR)
                            for i in (0, 1):
                                kc = 2 * m + i
                                dj = kc - qb * NDIAG
                                offc = P * dj if dj > 0 else 0
                                if not (USE_DR and USE_FP8_PROBS):
                                    nc.tensor.matmul(
                                        ps_d[:, offc:], ones_sb[:, 0, 0:1],
                                        e_pair[:, i, offc:],
                                        start=(st and i == 0),
                                        stop=(sp and i == 1))
                                nc.tensor.matmul(
                                    ps_o[:, offc:],
                                    v_sbuf[:, kc, h, :],
                                    e_pair[:, i, offc:],
                                    start=(st and i == 0),
                                    stop=(sp and i == 1))

                        e_prev = scores_exp(0)
                        for m in range(1, npair):
                            e_cur = scores_exp(m)
                            denom_pv(m - 1, e_prev)
                            oproj.step(hquota)
                            e_prev = e_cur
                        denom_pv(npair - 1, e_prev)
                        oproj.step(hquota)

                        inv_d = invp.tile([1, QB], F32, tag="inv")
                        nc.vector.reciprocal(inv_d[:], ps_d[:])
                        inv_b = invp.tile([P, QB], F32, tag="invb")
                        nc.gpsimd.partition_broadcast(inv_b[:], inv_d[:])
                        nc.vector.tensor_tensor(
                            attnT[:, h, qb * QB:(qb + 1) * QB],
                            ps_o[:],
                            inv_b[:],
                            MUL,
                        )
                    oproj.add_qb(qb, tail=(qb == NQB - 1))
                oproj.drain()

    nc.compile()
    return nc


def make_tables(token_positions, S_=S, DK=D_K):
    """Host-side RoPE tables (de-interleaved halves) + causal mask tiles."""
    pos = np.asarray(token_positions).astype(np.float64)
    half = np.arange(0, DK, 2, dtype=np.float64) / DK
    inv_freq = 1.0 / (ROPE_THETA ** half)  # [DK/2]
    ang = pos[:, None] * inv_freq[None, :]  # [S, DK/2]
    c = np.cos(ang).T.astype(np.float32)  # [DK/2, S]
    s = np.sin(ang).T.astype(np.float32)
    cosH = np.ascontiguousarray(np.concatenate([c, c], axis=0))  # [DK, S]
    sinH = np.ascontiguousarray(np.concatenate([s, s], axis=0))
    kl = np.arange(128)[:, None]
    ql = np.arange(128)[None, :]
    tri = (ql >= kl).astype(np.float32)  # [128, 128] causal triangle
    zt = np.concatenate([np.zeros_like(tri), tri], axis=1)  # [128, 256]
    masks = np.concatenate([tri, zt], axis=1)  # [128, 384]
    return cosH, sinH, masks


# de-interleave permutation within each head's 128 dims: even dims first
_DEINT = np.concatenate([np.arange(0, D_K, 2), np.arange(1, D_K, 2)])

BF16_NP = ml_dtypes.bfloat16
FP8_NP = ml_dtypes.float8_e4m3


def deinterleave_cols(wT, n_heads):
    """Permute per-head output columns of a [D, n_heads*DK] matrix so even
    RoPE dims land in rows 0..63 of the head-transposed projection."""
    w = np.asarray(wT)
    out = np.empty_like(w)
    for h in range(n_heads):
        out[:, h * D_K:(h + 1) * D_K] = w[:, h * D_K + _DEINT]
    return out


def make_in_maps(x, token_positions, q_w, k_w, v_w, o_w):
    cosH, sinH, masks = make_tables(token_positions)
    x = np.asarray(x, np.float32)
    in_maps = []
    for c in range(N_CORES):
        b, g = c // GROUPS, c % GROUPS
        e_lo, e_hi = g * H_LOC * D_K, (g + 1) * H_LOC * D_K
        wqT = np.asarray(q_w, np.float32)[e_lo:e_hi, :].T
        wkT = np.asarray(k_w, np.float32)[e_lo:e_hi, :].T
        in_maps.append({
            "xT": np.ascontiguousarray(x[b].T).astype(BF16_NP),
            "wqT": np.ascontiguousarray(
                deinterleave_cols(wqT, H_LOC)).astype(BF16_NP),
            "wkT": np.ascontiguousarray(
                deinterleave_cols(wkT, H_LOC)).astype(BF16_NP),
            "wvT": np.ascontiguousarray(
                np.asarray(v_w, np.float32)[e_lo:e_hi, :].T).astype(BF16_NP),
            "woT": np.ascontiguousarray(
                np.asarray(o_w, np.float32)[:, e_lo:e_hi].T).astype(BF16_NP),
            "cosH": cosH.astype(BF16_NP),
            "sinH": sinH.astype(BF16_NP),
            "masks": masks.astype(FP8_NP if USE_FP8_PROBS else BF16_NP),
            "ones": np.ones((128, 32), FP8_NP),
        })
    return in_maps


_NC_CACHE = None


def get_nc():
    global _NC_CACHE
    if _NC_CACHE is None:
        _NC_CACHE = build_nc(D_MODEL, S, H_LOC)
    return _NC_CACHE


def kernel(x, token_positions, q_w, k_w, v_w, o_w):
    from concourse.bass_utils import run_bass_kernel_spmd

    nc = get_nc()
    in_maps = make_in_maps(x, token_positions, q_w, k_w, v_w, o_w)
    res = run_bass_kernel_spmd(nc, in_maps, list(range(N_CORES)))
    outs = [np.asarray(res.results[c]["out"], dtype=np.float32)
            for c in range(N_CORES)]
    full = np.empty((B, S, D_MODEL), np.float32)
    for b in range(B):
        full[b] = outs[GROUPS * b]
        for g in range(1, GROUPS):
            full[b] += outs[GROUPS * b + g]
    return full


# revision 35
# speedup vs baseline: 1.0427x; 1.0427x over previous
"""Causal multi-head self-attention on 8 Trainium2 NeuronCores (v2).

Problem: B=4, S=2048, D_MODEL=2048, H=16 heads, d_k=128, RoPE, causal
softmax, fp32 I/O.

Sharding: 8 cores = 4 batches x 2 head-groups (8 heads each).  Each core
computes QKV projections for its head group (weights sharded by output
rows), RoPE, head-local causal attention, and a partial o_proj over its
1024 input features.  The host sums the two partial o_proj outputs per
batch.

v2 changes vs v1:
- All matmul operands bf16 (same 1 cyc/row PE rate as f32r, but no
  4x penalty on <256-wide moving suffixes, half the DMA bytes, half
  the SBUF), fp32 PSUM accumulation throughout.  fp8 was evaluated and
  rejected: probs overflow fp8e4's range on the real logit tails (std
  1.41, max 10.4 without max-subtraction), and fp8 V / o_proj measured
  over the 2e-2 error bar.
- Q/K/V never round-trip through DRAM: RoPE'd QT/KT and V (all bf16)
  stay SBUF-resident for the whole kernel.
- kc-chunks are processed in PAIRS; exp for non-diagonal pairs is one
  fused ACT op over both chunks' PSUM banks.
- o_proj matmuls (bf16) are spliced between attention pairs one q-block
  behind, so the PE never idles on the ACT exp latency; o_proj PSUM
  eviction runs on the (idle) DVE instead of ACT.
- x / weights stream on both HWDGE queues (SP + ACT) round-robin; out
  rides the HWDGE queues too (SWDGE ucode is ~1us/op on Pool).
- out is written bf16 (host upcasts; threshold is 2e-2 >> bf16 rounding).
"""

import sys

for _p in ("/opt/trn_rl_repo", "/root/.axon_site/_ro/trn_rl_repo"):
    if _p not in sys.path:
        sys.path.insert(0, _p)

import numpy as np
import ml_dtypes

import concourse.bacc as bacc
import concourse.mybir as mybir
import concourse.tile as tile

F32 = mybir.dt.float32
BF16 = mybir.dt.bfloat16
FP8 = mybir.dt.float8e4
EXPF = mybir.ActivationFunctionType.Exp
COPYF = mybir.ActivationFunctionType.Copy
MUL = mybir.AluOpType.mult
ADD = mybir.AluOpType.add
SUB = mybir.AluOpType.subtract
DR = mybir.MatmulPerfMode.DoubleRow

D_MODEL = 2048
NUM_HEADS = 16
D_K = 128
ROPE_THETA = 10000.0
B = 4
S = 2048
N_CORES = 8
GROUPS = 2  # head groups (tensor parallel factor)
H_LOC = NUM_HEADS // GROUPS  # heads per core

# Global bias folded into the exp (cancels in the softmax ratio).  The
# real logits here have std ~1.41 and max ~10.4: the fp8e4 dynamic range
# (~11.7 nats) cannot span the ~19-nat logit spread without max
# subtraction, so probs stay bf16 (no overflow: bf16 spans e+-88 nats).
EXP_BIAS = 0.0

# fp8 probs overflowed to Inf on the real logit tails (max logit 10.4 >
# ln(240)+bias headroom); V / o_proj in fp8 measured over the 2e-2 bar.
# Everything stays bf16.
USE_FP8_PROBS = False
USE_DR = False  # fp8 DoubleRow denominator requires fp8 probs


def build_nc(D=D_MODEL, S_=S, H_loc=H_LOC, QB=512):
    """Build the per-core Bass program. Parametric for small-size sim tests."""
    P = 128
    DK = 128
    HH = DK // 2
    E = H_loc * DK  # local qkv output features
    KCN = D // P  # contraction chunks for projections
    NSB = S_ // P  # 128-token blocks
    NQB = S_ // QB  # q blocks in attention
    NDIAG = QB // P  # diagonal 128-k chunks per q block (4)
    NST = max(1, S_ // 512)  # 512-wide s tiles in projections
    STW = S_ // NST
    NTW = min(512, D)  # o_proj output tile width
    NTN = D // NTW
    SCALE = 1.0 / float(np.sqrt(DK))

    nc = bacc.Bacc("TRN2", target_bir_lowering=False, debug=False,
                   num_devices=N_CORES)

    if (F32, EXP_BIAS) not in nc.const_aps.aps:
        # register the exp-bias const AP (only 0.0/1.0 ship by default)
        _bias_t = nc.alloc_sbuf_tensor("const-f32-expbias", [128, 1], F32)
        nc.gpsimd.memset(_bias_t.ap(), EXP_BIAS)
        nc.const_aps.aps[(F32, EXP_BIAS)] = _bias_t.ap()
        nc.all_engine_barrier()

    xT = nc.dram_tensor("xT", [D, S_], BF16, kind="ExternalInput")
    wqT = nc.dram_tensor("wqT", [D, E], BF16, kind="ExternalInput")
    wkT = nc.dram_tensor("wkT", [D, E], BF16, kind="ExternalInput")
    wvT = nc.dram_tensor("wvT", [D, E], BF16, kind="ExternalInput")
    woT = nc.dram_tensor("woT", [E, D], BF16, kind="ExternalInput")
    # RoPE tables for the DE-INTERLEAVED head layout (even dims in rows
    # 0..63, odd dims in rows 64..127; wq/wk columns permuted host-side,
    # leaving Q.K dot products invariant).  Duplicated to full d_k height
    # so both halves have base-0 AND base-64 slices.
    cosH = nc.dram_tensor("cosH", [DK, S_], BF16, kind="ExternalInput")
    sinH = nc.dram_tensor("sinH", [DK, S_], BF16, kind="ExternalInput")
    # masks[:, 0:128] = causal triangle; masks[:, 128:384] = [zeros | tri]
    # (the odd chunk of a diagonal pair needs a fully-invalid 128 block
    # followed by its triangle).
    EDT = FP8 if USE_FP8_PROBS else BF16  # probs dtype
    masks = nc.dram_tensor("masks", [P, 3 * P], EDT, kind="ExternalInput")
    # padded to [P, 2, 16] so the dual-fp8 ldweights sees a 16B-aligned
    # even step on the k-tile dim (s3_lw_dual_fp8_restrictions)
    ones_in = nc.dram_tensor("ones", [P, 32], FP8, kind="ExternalInput")
    out = nc.dram_tensor("out", [S_, D], BF16, kind="ExternalOutput")

    xT_t = xT.rearrange("(kc p) s -> p kc s", p=P)
    wT_t = {
        "q": wqT.rearrange("(kc p) e -> p kc e", p=P),
        "k": wkT.rearrange("(kc p) e -> p kc e", p=P),
        "v": wvT.rearrange("(kc p) e -> p kc e", p=P),
    }
    woT_t = woT.rearrange("(ec p) n -> p ec n", p=P)

    with tile.TileContext(nc) as tc:
        with (
            tc.tile_pool(name="const", bufs=1) as const,
            tc.tile_pool(name="qkv", bufs=1) as qkvp,
        ):
            mask_sb = const.tile([P, 3 * P], EDT)
            ones_sb = const.tile([P, 2, 16], FP8)
            cos_sb = const.tile([DK, S_], BF16)
            sin_sb = const.tile([DK, S_], BF16)
            # const loads ride the (idle) SWDGE so the HWDGE queues start
            # streaming x immediately
            nc.gpsimd.dma_start(mask_sb[:], masks[:])
            nc.gpsimd.dma_start(ones_sb[:], ones_in[:])

            # Q/K head-transposed [dk, h, s] bf16; V [s, sb, h, dk] bf16 --
            # SBUF-resident for the whole kernel (no DRAM roundtrip).
            qt_sbuf = qkvp.tile([DK, H_loc, S_], BF16)
            kt_sbuf = qkvp.tile([DK, H_loc, S_], BF16)
            v_sbuf = qkvp.tile([P, NSB, H_loc, DK], BF16)

            queues = [nc.sync, nc.scalar]  # two HWDGE queues

            # ---------------- Phase 1: projections + RoPE -----------------
            with tc.tile_pool(name="xres", bufs=1) as xres:
                x_res = xres.tile([P, KCN, S_], BF16)

                # --- Phase 1b: V via x-stationary matmuls -> v_sbuf fp8 ---
                with (
                    tc.tile_pool(name="wv", bufs=1) as wvp,
                    tc.tile_pool(name="v_ps", bufs=8, space="PSUM") as v_ps,
                ):
                    EH = min(512, E)
                    NH_H = EH // DK  # heads per feature-half
                    gsz = min(4, NSB)
                    for half in range(E // EH):
                        e_lo = half * EH
                        h_lo = e_lo // DK
                        wv_res = wvp.tile([P, KCN, EH], BF16, tag="wv",
                                          name=f"wv_{half}")
                        if half != 0:
                            for kc in range(KCN):
                                queues[kc % 2].dma_start(
                                    wv_res[:, kc],
                                    wT_t["v"][:, kc, e_lo:e_lo + EH],
                                )
                        sb0 = 0
                        for gi in range(NSB // gsz):
                            psv = [
                                v_ps.tile([P, EH], F32, tag="vps",
                                          name=f"vps_{half}_{gi}_{i}")
                                for i in range(gsz)
                            ]
                            for kc in range(KCN):
                                if half == 0 and gi == 0:
                                    # stream x + wv together on both queues;
                                    # first x chunk split across both queues
                                    # for startup latency
                                    if kc == 0:
                                        hw = S_ // 2
                                        queues[0].dma_start(
                                            x_res[:, 0, :hw],
                                            xT_t[:, 0, :hw])
                                        queues[1].dma_start(
                                            x_res[:, 0, hw:],
                                            xT_t[:, 0, hw:])
                                        nc.gpsimd.dma_start(
                                            wv_res[:, 0],
                                            wT_t["v"][:, 0,
                                                      e_lo:e_lo + EH])
                                    else:
                                        queues[kc % 2].dma_start(
                                            x_res[:, kc], xT_t[:, kc])
                                        queues[(kc + 1) % 2].dma_start(
                                            wv_res[:, kc],
                                            wT_t["v"][:, kc,
                                                      e_lo:e_lo + EH])
                                for i in range(gsz):
                                    sb_i = sb0 + i
                                    nc.tensor.matmul(
                                        psv[i][:],
                                        x_res[:, kc, sb_i * P:(sb_i + 1) * P],
                                        wv_res[:, kc],
                                        start=(kc == 0),
                                        stop=(kc == KCN - 1),
                                    )
                            for i in range(gsz):
                                sb_i = sb0 + i
                                nc.scalar.activation(
                                    v_sbuf[:, sb_i, h_lo:h_lo + NH_H, :],
                                    psv[i][:].rearrange(
                                        "p (a b) -> p a b", a=NH_H),
                                    COPYF,
                                )
                            sb0 += gsz
                    # trig tables land long before the first RoPE needs them
                    queues[0].dma_start(cos_sb[:], cosH[:])
                    queues[1].dma_start(sin_sb[:], sinH[:])

                # --- Phase 1a: Q/K head-transposed projections + RoPE ---
                with (
                    tc.tile_pool(name="wslice", bufs=6) as wslice,
                    tc.tile_pool(name="qk_ps", bufs=2, space="PSUM") as qk_ps,
                    tc.tile_pool(name="rawp", bufs=2) as rawp,
                    tc.tile_pool(name="ropet", bufs=2) as ropet,
                ):
                    qi = 0
                    for t in ("q", "k"):
                        for h in range(H_loc):
                            pgrp = qk_ps.tile([P, NST, STW], F32, tag="qk",
                                              name=f"pg_{t}_{h}")
                            for kc in range(KCN):
                                w_sl = wslice.tile([P, DK], BF16, tag="wsl")
                                queues[qi % 2].dma_start(
                                    w_sl[:],
                                    wT_t[t][:, kc, h * DK:(h + 1) * DK])
                                qi += 1
                                for st in range(NST):
                                    nc.tensor.matmul(
                                        pgrp[:, st],
                                        w_sl[:],
                                        x_res[:, kc,
                                              st * STW:(st + 1) * STW],
                                        start=(kc == 0),
                                        stop=(kc == KCN - 1),
                                    )
                            # single ACT eviction to bf16, then RoPE on DVE
                            # (bf16 = 2x DVE rate) writing straight into
                            # qt_sbuf/kt_sbuf.
                            raw = rawp.tile([DK, S_], BF16, tag="raw")
                            raw_v = raw[:].rearrange("p (a b) -> p a b",
                                                     b=STW)
                            nc.scalar.activation(raw_v, pgrp[:], COPYF)
                            dst = qt_sbuf if t == "q" else kt_sbuf
                            dstE = dst[:HH, h]
                            dstO = dst[HH:, h]
                            tmp = ropet.tile([HH, S_], BF16, tag="tmp")
                            rotO = ropet.tile([HH, S_], BF16, tag="rotO")
                            # rot_E = E*cos - O*sin (in-place in qt/kt)
                            nc.vector.tensor_tensor(
                                dstE, raw[:HH], cos_sb[:HH], MUL)
                            nc.vector.tensor_tensor(
                                tmp[:], raw[HH:], sin_sb[HH:], MUL)
                            nc.vector.tensor_tensor(
                                dstE, dstE, tmp[:], SUB)
                            # rot_O = E*sin + O*cos (via base-0 temps; the
                            # SB-SB input pair must share a base partition)
                            nc.vector.tensor_tensor(
                                rotO[:], raw[:HH], sin_sb[:HH], MUL)
                            nc.vector.tensor_tensor(
                                tmp[:], raw[HH:], cos_sb[HH:], MUL)
                            nc.vector.tensor_tensor(
                                dstO, rotO[:], tmp[:], ADD)

            # ---------- Phase 2+3: attention with o_proj spliced ----------
            with (
                tc.tile_pool(name="attnTp", bufs=1) as attnTp,
                tc.tile_pool(name="wop", bufs=1) as wop,
                tc.tile_pool(name="expt", bufs=4) as expt,
                tc.tile_pool(name="invp", bufs=2) as invp,
                tc.tile_pool(name="osb", bufs=3) as osbp,
                tc.tile_pool(name="sc_ps", bufs=2, space="PSUM") as sc_ps,
                tc.tile_pool(name="den_ps", bufs=1, space="PSUM") as den_ps,
                tc.tile_pool(name="pv_ps", bufs=1, space="PSUM") as pv_ps,
                tc.tile_pool(name="op_ps", bufs=2, space="PSUM") as op_ps,
            ):
                attnT = attnTp.tile([DK, H_loc, S_], BF16)
                wo_sb = wop.tile([P, H_loc, D], BF16)
                for ec in range(H_loc):
                    queues[ec % 2].dma_start(wo_sb[:, ec], woT_t[:, ec])

                mask_tri = mask_sb[:, 0:P]
                mask_zt = mask_sb[:, P:3 * P]

                # o_proj work list; units are spliced between attention
                # pairs one q-block behind so the PE never waits on exp.
                class OProj:
                    def __init__(self):
                        self.units = []
                        self.credit = 0.0

                    def add_qb(self, qb, tail=False):
                        for sb_i in range(qb * (QB // P), (qb + 1) * (QB // P)):
                            for nt in range(NTN):
                                ps = op_ps.tile([P, NTW], F32, tag="op",
                                                name=f"op_{sb_i}_{nt}")

                                for ec0 in range(0, H_loc, 2):
                                    def mm(sb_i=sb_i, nt=nt, ps=ps, ec0=ec0):
                                        for ec in (ec0, ec0 + 1):
                                            nc.tensor.matmul(
                                                ps[:],
                                                attnT[:, ec,
                                                      sb_i * P:(sb_i + 1) * P],
                                                wo_sb[:, ec,
                                                      nt * NTW:(nt + 1) * NTW],
                                                start=(ec == 0),
                                                stop=(ec == H_loc - 1),
                                            )
                                    self.units.append(mm)

                                # evict on DVE (ACT is busy with exp); in the
                                # tail drain ACT is idle, so alternate
                                alt = (sb_i + nt) % 2
                                def ev(sb_i=sb_i, nt=nt, ps=ps, alt=alt,
                                       tail=tail):
                                    o_nt = osbp.tile([P, NTW], BF16,
                                                     tag="osb")
                                    if tail and alt:
                                        nc.scalar.activation(
                                            o_nt[:], ps[:], COPYF)
                                    else:
                                        nc.vector.tensor_scalar_mul(
                                            o_nt[:], ps[:], 1.0)
                                    # out rides the HWDGE queues (idle in
                                    # the attention phase)
                                    queues[alt].dma_start(
                                        out[sb_i * P:(sb_i + 1) * P,
                                            nt * NTW:(nt + 1) * NTW],
                                        o_nt[:],
                                    )
                                self.units.append(ev)

                    def step(self, quota):
                        self.credit += quota
                        while self.credit >= 1.0 and self.units:
                            self.units.pop(0)()
                            self.credit -= 1.0

                    def drain(self):
                        while self.units:
                            self.units.pop(0)()

                oproj = OProj()

                for qb in range(NQB):
                    npair = (qb + 1) * NDIAG // 2
                    # warmup-skip: no filler during the first head of a qb
                    # (its attnT inputs are still in the normalize chain of
                    # the previous qb), redistribute over the rest
                    quota = (len(oproj.units) / float((H_loc - 1) * npair)
                             if qb > 0 and H_loc > 1 else 0.0)
                    for h in range(H_loc):
                        hquota = quota if (qb > 0 and h > 0) else 0.0
                        ps_d = den_ps.tile([1, QB], F32, tag="den")
                        ps_o = pv_ps.tile([P, QB], F32, tag="pv")

                        def off_of(m):
                            dj = m - qb * (NDIAG // 2)
                            return 2 * P * dj if dj > 0 else 0

                        def scores_exp(m):
                            off = off_of(m)
                            dj = m - qb * (NDIAG // 2)
                            e_pair = expt.tile([P, 2, QB], EDT, tag="e",
                                               name=f"e_{h}_{qb}_{m}")
                            ps_s = sc_ps.tile([P, 2, QB], F32, tag="sc",
                                              name=f"ss_{h}_{qb}_{m}")
                            for i in (0, 1):
                                kc = 2 * m + i
                                nc.tensor.matmul(
                                    ps_s[:, i, off:],
                                    kt_sbuf[:, h, kc * P:(kc + 1) * P],
                                    qt_sbuf[:, h, qb * QB + off:(qb + 1) * QB],
                                    start=True, stop=True,
                                )
                            if dj < 0:
                                # full pair: one fused exp over both chunks
                                nc.scalar.activation(
                                    e_pair[:].rearrange("p a b -> p (a b)"),
                                    ps_s[:].rearrange("p a b -> p (a b)"),
                                    EXPF, scale=SCALE, bias=EXP_BIAS)
                            else:
                                for i in (0, 1):
                                    nc.scalar.activation(
                                        e_pair[:, i, off:], ps_s[:, i, off:],
                                        EXPF, scale=SCALE, bias=EXP_BIAS)
                                # even chunk: triangle at [off, off+P);
                                # odd chunk: zeros [off,off+P) + triangle
                                nc.vector.tensor_tensor(
                                    e_pair[:, 0, off:off + P],
                                    e_pair[:, 0, off:off + P],
                                    mask_tri, MUL)
                                nc.vector.tensor_tensor(
                                    e_pair[:, 1, off:off + 2 * P],
                                    e_pair[:, 1, off:off + 2 * P],
                                    mask_zt, MUL)
                            return e_pair

                        def denom_pv(m, e_pair):
                            off = off_of(m)
                            st, sp = (m == 0), (m == npair - 1)
                            if USE_DR and USE_FP8_PROBS:
                                # fp8 DoubleRow denominator: 256-contraction
                                # per pass over the chunk pair
                                nc.tensor.matmul(
                                    ps_d[:, off:], ones_sb[:, :, 0:1],
                                    e_pair[:, :, off:],
                                    start=st, stop=sp, perf_mode=DR)
                            for i in (0, 1):
                                kc = 2 * m + i
                                dj = kc - qb * NDIAG
                                offc = P * dj if dj > 0 else 0
                                if not (USE_DR and USE_FP8_PROBS):
                                    nc.tensor.matmul(
                                        ps_d[:, offc:], ones_sb[:, 0, 0:1],
                                        e_pair[:, i, offc:],
                                        start=(st and i == 0),
                                        stop=(sp and i == 1))
                                nc.tensor.matmul(
                                    ps_o[:, offc:],
                                    v_sbuf[:, kc, h, :],
                                    e_pair[:, i, offc:],
                                    start=(st and i == 0),
                                    stop=(sp and i == 1))

                        # exp pipelined two pairs ahead: dp(m) trails
                        # exp(m) by two pairs of PE work so the ACT
                        # latency never stalls the PE
                        es = [scores_exp(0)]
                        if npair > 1:
                            es.append(scores_exp(1))
                        for m in range(npair):
                            if m + 2 < npair:
                                es.append(scores_exp(m + 2))
                            denom_pv(m, es[m])
                            oproj.step(hquota)

                        inv_d = invp.tile([1, QB], F32, tag="inv")
                        nc.vector.reciprocal(inv_d[:], ps_d[:])
                        inv_b = invp.tile([P, QB], F32, tag="invb")
                        nc.gpsimd.partition_broadcast(inv_b[:], inv_d[:])
                        nc.vector.tensor_tensor(
                            attnT[:, h, qb * QB:(qb + 1) * QB],
                            ps_o[:],
                            inv_b[:],
                            MUL,
                        )
                    oproj.add_qb(qb, tail=(qb == NQB - 1))
                oproj.drain()

    nc.compile()
    return nc


def make_tables(token_positions, S_=S, DK=D_K):
    """Host-side RoPE tables (de-interleaved halves) + causal mask tiles."""
    pos = np.asarray(token_positions).astype(np.float64)
    half = np.arange(0, DK, 2, dtype=np.float64) / DK
    inv_freq = 1.0 / (ROPE_THETA ** half)  # [DK/2]
    ang = pos[:, None] * inv_freq[None, :]  # [S, DK/2]
    c = np.cos(ang).T.astype(np.float32)  # [DK/2, S]
    s = np.sin(ang).T.astype(np.float32)
    cosH = np.ascontiguousarray(np.concatenate([c, c], axis=0))  # [DK, S]
    sinH = np.ascontiguousarray(np.concatenate([s, s], axis=0))
    kl = np.arange(128)[:, None]
    ql = np.arange(128)[None, :]
    tri = (ql >= kl).astype(np.float32)  # [128, 128] causal triangle
    zt = np.concatenate([np.zeros_like(tri), tri], axis=1)  # [128, 256]
    masks = np.concatenate([tri, zt], axis=1)  # [128, 384]
    return cosH, sinH, masks


# de-interleave permutation within each head's 128 dims: even dims first
_DEINT = np.concatenate([np.arange(0, D_K, 2), np.arange(1, D_K, 2)])

BF16_NP = ml_dtypes.bfloat16
FP8_NP = ml_dtypes.float8_e4m3


def deinterleave_cols(wT, n_heads):
    """Permute per-head output columns of a [D, n_heads*DK] matrix so even
    RoPE dims land in rows 0..63 of the head-transposed projection."""
    w = np.asarray(wT)
    out = np.empty_like(w)
    for h in range(n_heads):
        out[:, h * D_K:(h + 1) * D_K] = w[:, h * D_K + _DEINT]
    return out


def make_in_maps(x, token_positions, q_w, k_w, v_w, o_w):
    cosH, sinH, masks = make_tables(token_positions)
    x = np.asarray(x, np.float32)
    in_maps = []
    for c in range(N_CORES):
        b, g = c // GROUPS, c % GROUPS
        e_lo, e_hi = g * H_LOC * D_K, (g + 1) * H_LOC * D_K
        wqT = np.asarray(q_w, np.float32)[e_lo:e_hi, :].T
        wkT = np.asarray(k_w, np.float32)[e_lo:e_hi, :].T
        in_maps.append({
            "xT": np.ascontiguousarray(x[b].T).astype(BF16_NP),
            "wqT": np.ascontiguousarray(
                deinterleave_cols(wqT, H_LOC)).astype(BF16_NP),
            "wkT": np.ascontiguousarray(
                deinterleave_cols(wkT, H_LOC)).astype(BF16_NP),
            "wvT": np.ascontiguousarray(
                np.asarray(v_w, np.float32)[e_lo:e_hi, :].T).astype(BF16_NP),
            "woT": np.ascontiguousarray(
                np.asarray(o_w, np.float32)[:, e_lo:e_hi].T).astype(BF16_NP),
            "cosH": cosH.astype(BF16_NP),
            "sinH": sinH.astype(BF16_NP),
            "masks": masks.astype(FP8_NP if USE_FP8_PROBS else BF16_NP),
            "ones": np.ones((128, 32), FP8_NP),
        })
    return in_maps


_NC_CACHE = None


def get_nc():
    global _NC_CACHE
    if _NC_CACHE is None:
        _NC_CACHE = build_nc(D_MODEL, S, H_LOC)
    return _NC_CACHE


def kernel(x, token_positions, q_w, k_w, v_w, o_w):
    from concourse.bass_utils import run_bass_kernel_spmd

    nc = get_nc()
    in_maps = make_in_maps(x, token_positions, q_w, k_w, v_w, o_w)
    res = run_bass_kernel_spmd(nc, in_maps, list(range(N_CORES)))
    outs = [np.asarray(res.results[c]["out"], dtype=np.float32)
            for c in range(N_CORES)]
    full = np.empty((B, S, D_MODEL), np.float32)
    for b in range(B):
        full[b] = outs[GROUPS * b]
        for g in range(1, GROUPS):
            full[b] += outs[GROUPS * b + g]
    return full
